# revision 1
# baseline (speedup 1.0000x reference)
"""YOLO-head decode (nms_detection) on Trainium2.

Data-parallel over the batch dim: 16 batches -> 2 per core x 8 NeuronCores.
Per-core layout (128 partitions x 394 cells x 85 channels): the three
feature maps of both batches are regrouped host-side into fm-pure regions
  fm0: 38400 cells = 128 x 300   slots [  0,300)  scale W=80
  fm1:  9600 cells = 128 x  75   slots [300,375)  scale W=40
  fm2:  2400 cells -> 2432 = 128 x 19 (32 pad)  slots [375,394)  W=20
so every chunk sits in one region and its box scale is a compile-time
immediate.  The box columns are also staged host-side as a separate
contiguous f32 input so the exact-math path needs no strided (2x-penalty)
DMA.

Engine plan (per core, v1 cost model):
  Pool  fp8e4 cast loads of the full rows (~13us; conf/cls only need
        ~1e-2 rel err and sigmoid outputs are bounded away from 0) then
        an interleaved share of the stores.
  SP    contiguous f32 box block loads upfront (~2.5us) then the bulk of
        the stores.
  ACT   one warmup sigmoid to absorb the 1.3us activation-table load,
        then a gap-free strided sigmoid stream fp8 -> f32 out tiles
        (cols 4:85), one store at the very end.
  DVE   box decode per chunk from the f32 block tiles into out cols 0:4,
        bit-faithful op order vs the reference (x1=(b0-b2/2)*W etc), so
        the cancellation-prone box coords stay f32-exact.
Stores are assigned per chunk to whichever engine's queue drains first
(hand-tuned against CoreSim).
"""

import json

import numpy as np

_N_CORES = 8
_B_PER_CORE = 2
_D = 85             # 5 + 80 channels per cell
_P = 128            # partitions
_S = 394            # slots (cells per partition): 300 + 75 + 19
_N_TOT = 25200
_FM = [19200, 4800, 1200]          # cells per fm per batch
_RSLOT = [300, 75, 19]             # slots per region
_RPAD = [0, 0, 32]                 # pad cells per region (both batches)

# region-aligned chunk schedule: (slot_offset, size, box scale)
_CHUNKS = ([(0, 18, 80.0), (18, 24, 80.0), (42, 30, 80.0), (72, 36, 80.0),
            (108, 38, 80.0), (146, 38, 80.0), (184, 38, 80.0),
            (222, 40, 80.0), (262, 38, 80.0),
            (300, 38, 40.0), (338, 37, 40.0), (375, 19, 20.0)])
# per-chunk store engine (hand-tuned vs CoreSim)
_STORES = ["sync", "sync", "sync", "gpsimd", "sync", "gpsimd", "sync",
           "gpsimd", "sync", "gpsimd", "sync", "scalar"]

_state = {}


def _build(chunks=None, store_engines=None, load_dt="float8e4",
           box_engine="sync", box_group=3, io_bufs=10, out_bufs=10,
           scr_bufs=4, warmup=True):
    import concourse.bass as bass
    import concourse.mybir as mybir
    from concourse.tile import TileContext

    MUL = mybir.AluOpType.mult
    ADD = mybir.AluOpType.add
    SIG = mybir.ActivationFunctionType.Sigmoid
    f32 = mybir.dt.float32
    i32 = mybir.dt.int32
    dt = getattr(mybir.dt, load_dt)

    if chunks is None:
        chunks = list(_CHUNKS)
    if store_engines is None:
        store_engines = list(_STORES)
    n = len(chunks)
    assert len(store_engines) == n
    Kmax = max(K for _, K, _ in chunks)

    nc = bass.Bass()
    x = nc.dram_tensor("x", [_P, _S * _D], f32, kind="ExternalInput")
    xb = nc.dram_tensor("xb", [_P, _S * 4], f32, kind="ExternalInput")
    out = nc.dram_tensor("out", [_P, _S * _D], f32, kind="ExternalOutput")

    # box blocks: chunk 0 solo (ready earliest), then groups of box_group
    blocks = [(0, 1)]
    ci = 1
    while ci < n:
        blocks.append((ci, min(ci + box_group, n)))
        ci += box_group
    chunk_block = {}
    for bi, (c0, c1) in enumerate(blocks):
        for c in range(c0, c1):
            chunk_block[c] = bi

    with TileContext(nc) as tc:
        with (
            tc.tile_pool(name="const", bufs=1) as cp,
            tc.tile_pool(name="io", bufs=io_bufs) as iop,
            tc.tile_pool(name="op", bufs=out_bufs) as outp,
            tc.tile_pool(name="bx", bufs=1) as bxp,
            tc.tile_pool(name="scr", bufs=scr_bufs) as sp_,
        ):
            if warmup:
                # absorb the sigmoid activation-table load before the pipe
                idx = cp.tile([_P, 1], i32, name="idx")
                nc.gpsimd.iota(idx[:], pattern=[[1, 1]], base=0,
                               channel_multiplier=1)
                idxf = cp.tile([_P, 1], f32, name="idxf")
                nc.vector.tensor_copy(out=idxf[:], in_=idx[:])
                wrm = cp.tile([_P, 1], f32, name="wrm")
                nc.scalar.activation(wrm[:], idxf[:], SIG)

            blk_tiles = []
            for bi, (c0, c1) in enumerate(blocks):
                o0 = min(chunks[c][0] for c in range(c0, c1))
                o1 = max(chunks[c][0] + chunks[c][1] for c in range(c0, c1))
                bt = bxp.tile([_P, (o1 - o0) * 4], f32, tag=f"bx{bi}",
                              name=f"bx{bi}")
                getattr(nc, box_engine).dma_start(
                    out=bt[:], in_=xb[:, o0 * 4:o1 * 4])
                blk_tiles.append((bt, o0))

            pool_stores, act_stores = [], []
            for ci, (o, K, w) in enumerate(chunks):
                tl = iop.tile([_P, K * _D], dt, tag="io", name="io",
                              padded_shape=[_P, Kmax * _D])
                nc.gpsimd.dma_start(out=tl[:], in_=x[:, o * _D:(o + K) * _D])
                tv = tl.rearrange("p (k c) -> p k c", c=_D)

                ot = outp.tile([_P, K * _D], f32, tag="ot", name="ot",
                               padded_shape=[_P, Kmax * _D])
                ov = ot.rearrange("p (k c) -> p k c", c=_D)

                # DVE: box decode (bit-faithful op order vs the reference)
                bt, o0 = blk_tiles[chunk_block[ci]]
                r = o - o0
                bv = bt.rearrange("p (k c) -> p k c", c=4)
                r0, r1, r2, r3 = (bv[:, r:r + K, j] for j in range(4))
                h2 = sp_.tile([_P, K], f32, tag="h2", name="h2", padded_shape=[_P, Kmax])
                h3 = sp_.tile([_P, K], f32, tag="h3", name="h3", padded_shape=[_P, Kmax])
                u = sp_.tile([_P, K], f32, tag="u", name="u", padded_shape=[_P, Kmax])
                q = sp_.tile([_P, K], f32, tag="q", name="q", padded_shape=[_P, Kmax])
                t1 = sp_.tile([_P, K], f32, tag="t1", name="t1", padded_shape=[_P, Kmax])
                t2 = sp_.tile([_P, K], f32, tag="t2", name="t2", padded_shape=[_P, Kmax])
                nc.vector.tensor_scalar_mul(h2[:], r2, 0.5)
                nc.vector.tensor_scalar_mul(h3[:], r3, 0.5)
                nc.vector.tensor_sub(u[:], r0, h2[:])
                nc.vector.tensor_sub(q[:], r1, h3[:])
                nc.vector.tensor_scalar_mul(ov[:, :, 0], u[:], w)   # x1
                nc.vector.tensor_scalar_mul(ov[:, :, 1], q[:], w)   # y1
                nc.vector.scalar_tensor_tensor(t1[:], u[:], w, h2[:], op0=MUL, op1=ADD)
                nc.vector.tensor_scalar_mul(ov[:, :, 2], t1[:], w)  # x2
                nc.vector.scalar_tensor_tensor(t2[:], q[:], w, h3[:], op0=MUL, op1=ADD)
                nc.vector.tensor_scalar_mul(ov[:, :, 3], t2[:], w)  # y2

                # ACT: strided sigmoid fp8 -> f32 on conf+cls
                nc.scalar.activation(ov[:, :, 4:_D], tv[:, :, 4:_D], SIG)

                st = store_engines[ci]
                dst = out[:, o * _D:(o + K) * _D]
                if st == "gpsimd":
                    pool_stores.append((ot, dst))
                elif st == "scalar":
                    act_stores.append((ot, dst))
                else:
                    nc.sync.dma_start(out=dst, in_=ot[:])
            # Pool stores go after its load stream, ACT stores after the
            # sigmoid stream, so neither blocks its pipeline role.
            for ot, dst in pool_stores:
                nc.gpsimd.dma_start(out=dst, in_=ot[:])
            for ot, dst in act_stores:
                nc.scalar.dma_start(out=dst, in_=ot[:])

    return nc


def _split_multiwait_bir(bir_json):
    """Walrus codegen accepts a single sync-wait per instruction, but Tile's
    kernel-tail drain carries one wait per logical processor.  Split any
    multi-wait instruction into a chain of single-wait Drains on the same
    engine, keeping the last wait on the original instruction."""
    m = json.loads(bir_json)
    n = [0]

    def fix_block(b):
        insts = b.get("instructions") or []
        fixed = []
        for ins in insts:
            si = ins.get("sync_info") or {}
            waits = si.get("on_wait") or []
            if len(waits) > 1:
                for wt in waits[:-1]:
                    n[0] += 1
                    fixed.append({
                        "debug": ins.get("debug", 0),
                        "engine": ins["engine"],
                        "ins": [],
                        "name": f"I-waitsplit-{n[0]}",
                        "opcode": "Drain",
                        "outs": [],
                        "sync_info": {"on_update": [], "on_wait": [wt]},
                    })
                si["on_wait"] = [waits[-1]]
            fixed.append(ins)
        if insts:
            b["instructions"] = fixed
        for sb in b.get("blocks") or []:
            fix_block(sb)

    for fn in m["functions"]:
        for b in fn["blocks"]:
            fix_block(b)
    return json.dumps(m).encode()


def _install_bir_legalizer():
    if _state.get("patched"):
        return
    import concourse.bass2jax as bass2jax
    from concourse.bass_utils import compile_bir_kernel as orig

    def patched(bir_json, tmpdir, neff_name="file.neff"):
        return orig(_split_multiwait_bir(bir_json), tmpdir, neff_name)

    bass2jax.compile_bir_kernel = patched
    _state["patched"] = True


def _get_nc():
    if "nc" not in _state:
        _state["nc"] = _build()
    return _state["nc"]


def _pack(fm0, fm1, fm2):
    """[16,...] feature maps -> (x, xb): [8*128, S*85] and [8*128, S*4]."""
    fms = [fm0.reshape(16, -1, _D), fm1.reshape(16, -1, _D),
           fm2.reshape(16, -1, _D)]
    parts = []
    for r, fm in enumerate(fms):
        # per core: both batches' cells of this fm -> [8, 128, slots_r, 85]
        a = fm.reshape(_N_CORES, _B_PER_CORE * _FM[r], _D)
        if _RPAD[r]:
            a = np.concatenate(
                [a, np.zeros((_N_CORES, _RPAD[r], _D), a.dtype)], axis=1)
        parts.append(a.reshape(_N_CORES, _P, _RSLOT[r], _D))
    xfull = np.concatenate(parts, axis=2)          # [8, 128, 394, 85]
    xbfull = np.ascontiguousarray(xfull[..., 0:4])  # [8, 128, 394, 4]
    return (xfull.reshape(_N_CORES * _P, _S * _D),
            xbfull.reshape(_N_CORES * _P, _S * 4))


def _unpack(out):
    """[8*128, S*85] -> [16, 25200, 85]."""
    o = out.reshape(_N_CORES, _P, _S, _D)
    res = []
    off = 0
    for r in range(3):
        a = o[:, :, off:off + _RSLOT[r], :].reshape(_N_CORES, -1, _D)
        a = a[:, :_B_PER_CORE * _FM[r], :]
        res.append(a.reshape(_N_CORES * _B_PER_CORE, _FM[r], _D))
        off += _RSLOT[r]
    return np.concatenate(res, axis=1)             # [16, 25200, 85]


def _run_shards(fm0, fm1, fm2, **run_kwargs):
    from concourse.bass_utils import run_bass_kernel_spmd

    _install_bir_legalizer()
    nc = _get_nc()
    xfull, xbfull = _pack(fm0, fm1, fm2)
    in_maps = []
    for i in range(_N_CORES):
        in_maps.append({
            "x": xfull[_P * i:_P * (i + 1)],
            "xb": xbfull[_P * i:_P * (i + 1)],
        })
    res = run_bass_kernel_spmd(nc, in_maps, list(range(_N_CORES)), **run_kwargs)
    out = np.concatenate([r["out"] for r in res.results], axis=0)
    return _unpack(out.reshape(_N_CORES * _P, _S * _D))


def _direct_runner():
    """Direct shard_map runner over the prebuilt Bass module.  Equivalent to
    run_bass_kernel_spmd's axon path but feeds the packed full-batch arrays
    without the per-core split + re-concat, and keeps the (never-read,
    fully-overwritten) output buffers resident on device across calls."""
    if "direct" in _state:
        return _state["direct"]

    import jax
    import concourse.mybir as mybir
    from concourse.bass2jax import _bass_exec_p, partition_id_tensor
    from jax.sharding import Mesh, PartitionSpec, NamedSharding
    from jax.experimental.shard_map import shard_map

    _install_bir_legalizer()
    nc = _get_nc()
    partition_name = nc.partition_id_tensor.name if nc.partition_id_tensor else None
    out_avals = []
    for alloc in nc.m.functions[0].allocations:
        if not isinstance(alloc, mybir.MemoryLocationSet):
            continue
        if alloc.kind == "ExternalOutput":
            shape = tuple(alloc.tensor_shape)
            dtype = mybir.dt.np(alloc.dtype)
            out_avals.append(jax.core.ShapedArray(shape, dtype))
    in_names = ["x", "xb", "out"]
    if partition_name is not None:
        in_names.append(partition_name)

    def _body(*args):
        operands = list(args)
        if partition_name is not None:
            operands.append(partition_id_tensor())
        return tuple(_bass_exec_p.bind(
            *operands, out_avals=tuple(out_avals), in_names=tuple(in_names),
            out_names=("out",), lowering_input_output_aliases=(),
            sim_require_finite=True, sim_require_nnan=True, nc=nc))

    devices = jax.devices()[:_N_CORES]
    assert len(devices) == _N_CORES
    mesh = Mesh(np.asarray(devices), ("core",))
    spec = PartitionSpec("core")
    sharded = jax.jit(shard_map(
        _body, mesh=mesh, in_specs=(spec, spec, spec), out_specs=(spec,),
        check_rep=False))
    sh = NamedSharding(mesh, spec)
    dev_zeros = jax.device_put(
        np.zeros((_N_CORES * _P, _S * _D), np.float32), sh)
    _state["direct"] = (sharded, dev_zeros)
    return _state["direct"]


def kernel(fm0, fm1, fm2, detection_targets=None, **_unused):
    fm0 = np.asarray(fm0, dtype=np.float32)
    fm1 = np.asarray(fm1, dtype=np.float32)
    fm2 = np.asarray(fm2, dtype=np.float32)
    try:
        xfull, xbfull = _pack(fm0, fm1, fm2)
        sharded, dev_zeros = _direct_runner()
        (out,) = sharded(xfull, xbfull, dev_zeros)
        return _unpack(np.asarray(out))
    except Exception:
        _state.pop("direct", None)
        return _run_shards(fm0, fm1, fm2)

